# revision 1
# baseline (speedup 1.0000x reference)
"""Deformable conv block (offset conv -> bilinear sample -> conv -> BN -> ReLU)
on 8 Trainium2 NeuronCores, data-parallel over batch.

Self-contained: hardcodes all shapes. kernel(**inputs) takes full inputs,
shards batch across 8 cores, runs one Bass/Tile SPMD program, returns the
full [8, 64, 96, 96] float32 output.

Per-core device pipeline:
  1. offset conv (3x3, fp32r matmuls, channel-major)
  2. PE-transpose offsets to pixel-major [128, 72]
  3. per tap: bilinear weights + gather index q (magic-number floor; zero
     padding handled by a host-built padded sample table -> no OOB masks)
  4. per tap: build int16 gather indices in the HW 16-row-wrap layout via
     PE double-transpose, replicate to all 8 row groups by DMA
  5. per (chunk, tap): dma_gather 1KB descriptors from x2pad windows
     (one descriptor = 4 bilinear corners x 64 channels), DVE weighting
     with per-pixel broadcast APs, PE pair-transposes to channel-major,
     fp32r conv matmuls accumulating over taps in PSUM
  6. BN stats (ACT accum_out) + AllReduce [64,2] across the 8 cores,
     scale/bias fold (conv bias b_def cancels), ReLU, un-permuting DMA out
"""
import os
from contextlib import ExitStack

import numpy as np

import concourse.bass as bass
import concourse.tile as tile
from concourse import bacc, mybir, bass_utils

dt = mybir.dt
AOT = mybir.AluOpType
AFT = mybir.ActivationFunctionType

# problem shapes
B, C, H, W, K = 8, 64, 96, 96, 3
HW = H * W                # 9216
K2 = K * K                # 9
NCORES = 8
EPS = 1e-5

# padded sample-grid geometry: padded coord = image coord + PADM
PADM = 3                  # margin for floor(py) in [0, 100]
PW = W + 2 * PADM + 1     # 103 padded grid width
PR = H + 2 * PADM + 1     # 103 padded grid rows
NQ = PW * PR              # 10609 rows in x2pad
QCLAMP = float(W + 2 * PADM - 2)  # 100: floor clamp ceiling

NB = HW // 128            # 72 pixel-major block columns
CHUNK = 12                # block columns per main-loop chunk
NCH = NB // CHUNK         # 6 chunks
NPIX = CHUNK * 128        # 1536 pixels per chunk
MAGIC = 8388608.0         # 2^23

_CACHE = {}


def _build(nc, ncores=NCORES, use_collective=True):
    STOP = os.environ.get("KSTOP", "full")
    xpad = nc.dram_tensor("xpad", [C, 98 * 98], dt.float32r, kind="ExternalInput").ap()
    x2pad = nc.dram_tensor("x2pad", [NQ, 128], dt.float32, kind="ExternalInput").ap()
    woffT = nc.dram_tensor("woffT", [C, K2 * 18], dt.float32r, kind="ExternalInput").ap()
    wdefT = nc.dram_tensor("wdefT", [128, K2 * C], dt.float32r, kind="ExternalInput").ap()
    bpy = nc.dram_tensor("bpy", [128, K2 * NB], dt.float32, kind="ExternalInput").ap()
    bpx = nc.dram_tensor("bpx", [128, K2 * NB], dt.float32, kind="ExternalInput").ap()
    ident = nc.dram_tensor("ident", [128, 128], dt.float32, kind="ExternalInput").ap()
    idxh = nc.dram_tensor("idxh", [128, K2 * (HW // 16)], dt.int16,
                          kind="ExternalInput").ap()
    bnc = nc.dram_tensor("bnc", [C, 2], dt.float32, kind="ExternalInput").ap()
    out_d = nc.dram_tensor("out", [C, HW], dt.float32, kind="ExternalOutput").ap()

    with tile.TileContext(nc) as tc:
        with ExitStack() as ctx:
            cpool = ctx.enter_context(tc.tile_pool(name="const", bufs=1))
            ppool = ctx.enter_context(tc.tile_pool(name="persist", bufs=1))
            spool = ctx.enter_context(tc.tile_pool(name="small", bufs=2))
            gpool = ctx.enter_context(tc.tile_pool(name="gather", bufs=2))
            wpool = ctx.enter_context(tc.tile_pool(name="work", bufs=2))
            dpool = ctx.enter_context(tc.tile_pool(name="dram", bufs=1, space="DRAM"))
            ps_m = ctx.enter_context(tc.tile_pool(name="ps_m", bufs=2, space="PSUM"))
            ps_t = ctx.enter_context(tc.tile_pool(name="ps_t", bufs=1, space="PSUM"))
            ps_o = ctx.enter_context(tc.tile_pool(name="ps_o", bufs=1, space="PSUM"))

            # ---- load constants ----
            woffT_s = cpool.tile([C, K2 * 18], dt.float32r)
            nc.sync.dma_start(woffT_s[:], woffT)
            wdefT_s = cpool.tile([128, K2 * C], dt.float32r)
            nc.sync.dma_start(wdefT_s[:], wdefT)
            bpy_s = cpool.tile([128, K2 * NB], dt.float32)
            nc.sync.dma_start(bpy_s[:], bpy)
            bpx_s = cpool.tile([128, K2 * NB], dt.float32)
            nc.sync.dma_start(bpx_s[:], bpx)
            id_s = cpool.tile([128, 128], dt.float32)
            nc.sync.dma_start(id_s[:], ident)
            bnc_s = cpool.tile([C, 2], dt.float32)
            nc.sync.dma_start(bnc_s[:], bnc)

            # ---- 1+2. offset conv (streamed) + transpose to pixel-major ----
            # offT_s[p, 18*b + j] = off[j, 128*b + p]
            offT_s = ppool.tile([128, NB * 18], dt.float32)
            xpv = xpad.rearrange("c (h w) -> c h w", w=98)
            for cc in range(24):            # chunks of 4 rows = 384 px = 3 blocks
                xpc = wpool.tile([C, 6 * 98], dt.float32r, tag="xpc")
                nc.sync.dma_start(
                    xpc[:].rearrange("c (h w) -> c h w", w=98),
                    xpv[:, 4 * cc: 4 * cc + 6, :])
                xv = xpc[:].rearrange("c (h w) -> c h w", w=98)
                po = ps_m.tile([18, 384], dt.float32, tag="ps_misc")
                for k in range(K2):
                    ky, kx = k // K, k % K
                    rhs = xv[:, ky: ky + 4, kx: kx + 96]
                    nc.tensor.matmul(po[:], woffT_s[:, 18 * k: 18 * k + 18], rhs,
                                     start=(k == 0), stop=(k == K2 - 1))
                offc = wpool.tile([18, 384], dt.float32, tag="offc")
                nc.scalar.copy(offc[:], po[:])
                for cb in range(3):
                    pt = ps_m.tile([128, 18], dt.float32, tag="ps_misc")
                    nc.tensor.transpose(pt[:], offc[:, 128 * cb: 128 * cb + 128],
                                        id_s[0:18, 0:18])
                    c = 3 * cc + cb
                    nc.vector.tensor_copy(offT_s[:, 18 * c: 18 * c + 18], pt[:])
            offT_v = offT_s[:].rearrange("p (b j) -> p b j", j=18)
            if STOP == "off":
                nc.sync.dma_start(out_d[:, 0:NB * 18].rearrange("c (a b) -> c a b", a=2),
                                  offT_s[:].rearrange("p (a b) -> p a b", a=2)[0:64])
                return

            # ---- 3+4. per tap: weights w4 + int16 idx (16-wrap, replicated) ----
            w4_all = ppool.tile([128, K2 * NB * 4], dt.float32)
            w4v_all = w4_all[:].rearrange("p (k b j u) -> p k b j u", k=K2, j=4, u=1)
            idx_all = ppool.tile([128, K2 * (HW // 16)], dt.int16)
            for k in range(K2):
                py = spool.tile([128, NB], dt.float32, tag="py")
                nc.vector.tensor_tensor(py[:], offT_v[:, :, 2 * k],
                                        bpy_s[:, NB * k: NB * k + NB], AOT.add)
                px = spool.tile([128, NB], dt.float32, tag="px")
                nc.vector.tensor_tensor(px[:], offT_v[:, :, 2 * k + 1],
                                        bpx_s[:, NB * k: NB * k + NB], AOT.add)
                ry = spool.tile([128, NB], dt.float32, tag="ry")
                nc.vector.tensor_scalar(ry[:], py[:], MAGIC - 0.5, None, AOT.add)
                fy = spool.tile([128, NB], dt.float32, tag="fy")
                nc.vector.tensor_scalar(fy[:], ry[:], MAGIC, None, AOT.subtract)
                rx = spool.tile([128, NB], dt.float32, tag="rx")
                nc.vector.tensor_scalar(rx[:], px[:], MAGIC - 0.5, None, AOT.add)
                fx = spool.tile([128, NB], dt.float32, tag="fx")
                nc.vector.tensor_scalar(fx[:], rx[:], MAGIC, None, AOT.subtract)
                ly = spool.tile([128, NB], dt.float32, tag="ly")
                nc.vector.tensor_tensor(ly[:], py[:], fy[:], AOT.subtract)
                lx = spool.tile([128, NB], dt.float32, tag="lx")
                nc.vector.tensor_tensor(lx[:], px[:], fx[:], AOT.subtract)
                wy0 = spool.tile([128, NB], dt.float32, tag="wy0")
                nc.vector.tensor_scalar(wy0[:], ly[:], -1.0, 1.0, AOT.mult, AOT.add)
                wx0 = spool.tile([128, NB], dt.float32, tag="wx0")
                nc.vector.tensor_scalar(wx0[:], lx[:], -1.0, 1.0, AOT.mult, AOT.add)
                # corner blocks in x2pad window order: [c00, c10, c01, c11]
                w4v = w4v_all[:, k]
                nc.vector.tensor_tensor(w4v[:, :, 0, 0], wy0[:], wx0[:], AOT.mult)
                nc.vector.tensor_tensor(w4v[:, :, 1, 0], ly[:], wx0[:], AOT.mult)
                nc.vector.tensor_tensor(w4v[:, :, 2, 0], wy0[:], lx[:], AOT.mult)
                nc.vector.tensor_tensor(w4v[:, :, 3, 0], ly[:], lx[:], AOT.mult)
                qy = spool.tile([128, NB], dt.float32, tag="qy")
                nc.vector.tensor_scalar(qy[:], fy[:], 0.0, QCLAMP, AOT.max, AOT.min)
                qx = spool.tile([128, NB], dt.float32, tag="qx")
                nc.vector.tensor_scalar(qx[:], fx[:], 0.0, QCLAMP, AOT.max, AOT.min)
                qf = spool.tile([128, NB], dt.float32, tag="qf")
                nc.vector.scalar_tensor_tensor(qf[:], qy[:], float(PW), qx[:],
                                               AOT.mult, AOT.add)
                # idx construction: T1 then 8x T2 into 16-row staging
                t1p = ps_m.tile([NB, 128], dt.float32, tag="ps_misc")
                nc.tensor.transpose(t1p[:], qf[:], id_s[:])
                t1s = spool.tile([NB, 128], dt.float32, tag="t1s")
                nc.vector.tensor_copy(t1s[:], t1p[:])
                stv = idx_all[0:16, :].rearrange("p (k b r) -> p k b r", k=K2, r=8)
                for r in range(8):
                    t2p = ps_m.tile([16, NB], dt.float32, tag="ps_misc")
                    nc.tensor.transpose(t2p[:], t1s[:, 16 * r: 16 * r + 16],
                                        id_s[0:NB, 0:NB])
                    nc.any.tensor_copy(stv[:, k, :, r], t2p[:])
                # replicate rows 0..15 -> groups 1..7 for this tap
                for g in range(1, 8):
                    nc.sync.dma_start(
                        idx_all[16 * g: 16 * g + 16,
                                (HW // 16) * k: (HW // 16) * (k + 1)],
                        idx_all[0:16, (HW // 16) * k: (HW // 16) * (k + 1)])

            if STOP == "idx":
                nc.sync.dma_start(out_d[0:64, 0:K2 * NB * 4], w4_all[0:64, :])
                nc.sync.dma_start(
                    out_d[0:64, K2 * NB * 4: K2 * NB * 4 + K2 * HW // 16].bitcast(dt.int16)[:, 0:K2 * HW // 16],
                    idx_all[0:64, :])
                return

            if STOP in ("g1h", "hostidx"):
                nc.sync.dma_start(idx_all[:], idxh)

            # ---- 5. main loop: gather -> weight -> transpose -> conv ----
            DO_W = STOP not in ("g1", "g1h")
            DO_T = STOP in ("gt", "nm", "loop", "full")
            DO_MM = STOP in ("loop", "full")
            NCH_RUN = 1 if STOP in ("g1", "g1h", "gw", "gt", "nm") else NCH
            x2win = bass.AP(x2pad.tensor, 0, [[128, NQ - 1], [1, 256]])
            conv_s = ppool.tile([C, HW], dt.float32)
            sums = ppool.tile([C, 8], dt.float32)
            sqs = ppool.tile([C, 8], dt.float32)
            for ch in range(NCH_RUN):
                # 4 bank-aligned accumulation groups: e1 e2 | o1 o2
                po = ps_o.tile([C, 2048], dt.float32, tag="ps_out")
                for k in range(K2):
                    g_t = gpool.tile([128, CHUNK * 256], dt.float32, tag="g")
                    gview = g_t[:].rearrange("p (b e) -> p b e", e=256)
                    for g3 in range(3):
                        cbase = (HW // 16) * k + 96 * ch + 32 * g3
                        nc.gpsimd.dma_gather(
                            out_ap=gview[:, 4 * g3: 4 * g3 + 4, :],
                            in_ap=x2win,
                            idxs_ap=idx_all[:, cbase: cbase + 32],
                            num_idxs=512,
                            num_idxs_reg=512,
                            elem_size=256,
                            elem_step=128,
                        )
                    if not DO_W:
                        nc.sync.dma_start(out_d[0:64, 0:CHUNK * 256], g_t[0:64, :])
                        return
                    gv = g_t[:].rearrange("p (b j c) -> p b j c", j=4, c=C)
                    s_t = wpool.tile([128, CHUNK * C], dt.float32, tag="s")
                    sv = s_t[:].rearrange("p (b c) -> p b c", c=C)
                    tmp = wpool.tile([128, CHUNK * C], dt.float32, tag="tmp")
                    tv = tmp[:].rearrange("p (b c) -> p b c", c=C)
                    for j in range(4):
                        wj = w4v_all[:, k, CHUNK * ch: CHUNK * ch + CHUNK, j, :]
                        a1, a2 = bass.broadcast_tensor_aps(gv[:, :, j, :], wj)
                        if j == 0:
                            nc.vector.tensor_tensor(sv, a1, a2, AOT.mult)
                        else:
                            nc.vector.tensor_tensor(tv, a1, a2, AOT.mult)
                            nc.vector.tensor_tensor(sv, sv, tv, AOT.add)
                    if not DO_T:
                        if k == K2 - 1:
                            nc.sync.dma_start(out_d[0:64, 0:CHUNK * C], s_t[0:64, :])
                            return
                        continue
                    # pair transposes -> channel-major sampled
                    pt = ps_t.tile([128, CHUNK * C], dt.float32, tag="ps_tr")
                    for bb in range(CHUNK // 2):
                        nc.tensor.transpose(pt[:, 128 * bb: 128 * bb + 128],
                                            s_t[:, 128 * bb: 128 * bb + 128],
                                            id_s[:])
                    samp = wpool.tile([128, CHUNK * C], dt.float32r, tag="samp")
                    nc.scalar.copy(samp[:], pt[:])
                    if not DO_MM:
                        if k == K2 - 1:
                            nc.sync.dma_start(out_d[0:64, 0:CHUNK * C], samp[0:64, :].bitcast(dt.float32))
                            return
                        continue
                    # conv matmuls: accumulate over taps, parity-major out cols
                    st, sp = (k == 0), (k == K2 - 1)
                    lhe = wdefT_s[0:64, C * k: C * k + C]
                    lho = wdefT_s[64:128, C * k: C * k + C]
                    nc.tensor.matmul(po[:, 0:512], lhe, samp[0:64, 0:512],
                                     start=st, stop=sp)
                    nc.tensor.matmul(po[:, 512:768], lhe, samp[0:64, 512:768],
                                     start=st, stop=sp)
                    nc.tensor.matmul(po[:, 1024:1536], lho, samp[64:128, 0:512],
                                     start=st, stop=sp)
                    nc.tensor.matmul(po[:, 1536:1792], lho, samp[64:128, 512:768],
                                     start=st, stop=sp)
                # copy conv chunk to SBUF + per-chunk sum / sumsq
                cview = conv_s[:, NPIX * ch: NPIX * ch + NPIX].rearrange(
                    "c (h x) -> c h x", x=768)
                pov = po[:].rearrange("c (h x) -> c h x", x=1024)[:, :, 0:768]
                nc.scalar.activation(cview, pov, AFT.Copy,
                                     accum_out=sums[:, ch: ch + 1])
                scr = wpool.tile([C, NPIX], dt.float32, tag="scr", bufs=1)
                nc.scalar.activation(scr[:], cview, AFT.Square,
                                     accum_out=sqs[:, ch: ch + 1])

            if STOP == "loop":
                nc.sync.dma_start(out_d[:], conv_s[:])
                return

            # ---- 6. BN stats allreduce + normalize + relu + output ----
            st2 = ppool.tile([C, 2], dt.float32)
            nc.vector.tensor_reduce(st2[:, 0:1], sums[:, 0:NCH],
                                    mybir.AxisListType.X, AOT.add)
            nc.vector.tensor_reduce(st2[:, 1:2], sqs[:, 0:NCH],
                                    mybir.AxisListType.X, AOT.add)
            bi = dpool.tile([C, 2], dt.float32)
            bo = dpool.tile([C, 2], dt.float32)
            nc.gpsimd.dma_start(bi[:], st2[:])
            if use_collective:
                nc.gpsimd.collective_compute(
                    "AllReduce", AOT.add,
                    replica_groups=[list(range(ncores))],
                    ins=[bi.opt()], outs=[bo.opt()])
            else:
                nc.gpsimd.dma_start(bo[:], bi[:])
            ast = ppool.tile([C, 2], dt.float32)
            nc.gpsimd.dma_start(ast[:], bo[:])

            inv_n = 1.0 / float(ncores * HW)
            mean = ppool.tile([C, 1], dt.float32)
            nc.vector.tensor_scalar(mean[:], ast[:, 0:1], inv_n, None, AOT.mult)
            msq = ppool.tile([C, 1], dt.float32)
            nc.vector.tensor_scalar(msq[:], ast[:, 1:2], inv_n, None, AOT.mult)
            m2 = ppool.tile([C, 1], dt.float32)
            nc.vector.tensor_tensor(m2[:], mean[:], mean[:], AOT.mult)
            var = ppool.tile([C, 1], dt.float32)
            nc.vector.tensor_tensor(var[:], msq[:], m2[:], AOT.subtract)
            vare = ppool.tile([C, 1], dt.float32)
            nc.vector.tensor_scalar(vare[:], var[:], EPS, None, AOT.add)
            sd = ppool.tile([C, 1], dt.float32)
            nc.scalar.activation(sd[:], vare[:], AFT.Sqrt)
            inv = ppool.tile([C, 1], dt.float32)
            nc.vector.reciprocal(inv[:], sd[:])
            scl = ppool.tile([C, 1], dt.float32)
            nc.vector.tensor_tensor(scl[:], bnc_s[:, 0:1], inv[:], AOT.mult)
            mt = ppool.tile([C, 1], dt.float32)
            nc.vector.tensor_tensor(mt[:], mean[:], scl[:], AOT.mult)
            bia = ppool.tile([C, 1], dt.float32)
            nc.vector.tensor_tensor(bia[:], bnc_s[:, 1:2], mt[:], AOT.subtract)

            ov = out_d.rearrange("c (n q) -> c n q", q=128)
            for ch in range(NCH):
                on = wpool.tile([C, NPIX], dt.float32, tag="on")
                nc.scalar.activation(on[:], conv_s[:, NPIX * ch: NPIX * ch + NPIX],
                                     AFT.Relu, bias=bia[:], scale=scl[:])
                onv = on[:].rearrange("c (n q) -> c n q", q=128)
                # even b -> pixel cols 1536ch + 256B + p ; odd -> +128
                nc.sync.dma_start(ov[:, 12 * ch: 12 * ch + 12: 2, :], onv[:, 0:6, :])
                nc.sync.dma_start(ov[:, 12 * ch + 1: 12 * ch + 12: 2, :], onv[:, 6:12, :])


def _prep_core(xb, w_off, b_off, w_def, gamma, beta):
    """Host-side input prep for one batch item. xb: [64, 96, 96] f32."""
    ins = {}
    # xpad: zero-pad by 1 for the 3x3 offset conv
    xp = np.zeros((C, 98, 98), np.float32)
    xp[:, 1:97, 1:97] = xb
    ins["xpad"] = xp.reshape(C, 98 * 98)
    # x2pad: padded HWC grid, rows [xz[r,s,:], xz[r+1,s,:]]
    xz = np.zeros((PR + 1, PW, C), np.float32)
    xz[PADM:PADM + H, PADM:PADM + W] = xb.transpose(1, 2, 0)
    xzf = xz.reshape((PR + 1) * PW, C)
    ins["x2pad"] = np.concatenate([xzf[0:NQ], xzf[PW:NQ + PW]], axis=1)
    # weight rearrangements
    wofft = np.zeros((C, K2 * 18), np.float32)
    for k in range(K2):
        wofft[:, 18 * k:18 * k + 18] = w_off[:, :, k // K, k % K].T
    ins["woffT"] = wofft
    wdeft = np.zeros((128, K2 * C), np.float32)
    for k in range(K2):
        blk = w_def[:, :, k // K, k % K].T  # [cin, cout]
        wdeft[0:64, C * k:C * k + C] = blk
        wdeft[64:128, C * k:C * k + C] = blk
    ins["wdefT"] = wdeft
    # base grids (pixel-major [128, 72] per tap), fold b_off and pad margin
    pixi = np.arange(HW, dtype=np.int64)
    ygrid = (pixi // W).astype(np.float32)
    xgrid = (pixi % W).astype(np.float32)
    ypm = ygrid.reshape(NB, 128).T    # [p, b] pixel-major
    xpm = xgrid.reshape(NB, 128).T
    bpy = np.zeros((128, K2 * NB), np.float32)
    bpx = np.zeros((128, K2 * NB), np.float32)
    for k in range(K2):
        ky, kx = k // K - 1, k % K - 1
        bpy[:, NB * k:NB * k + NB] = ypm + (ky + PADM + b_off[2 * k])
        bpx[:, NB * k:NB * k + NB] = xpm + (kx + PADM + b_off[2 * k + 1])
    ins["bpy"] = bpy
    ins["bpx"] = bpx
    ins["ident"] = np.eye(128, dtype=np.float32)
    ins["idxh"] = np.full((128, K2 * (HW // 16)), 5000, dtype=np.int16)
    ins["bnc"] = np.stack([gamma, beta], axis=1).astype(np.float32)
    return ins


def _get_nc():
    if "nc" not in _CACHE:
        nc = bacc.Bacc("TRN2", target_bir_lowering=False, debug=False,
                       num_devices=NCORES)
        _build(nc)
        nc.compile()
        _CACHE["nc"] = nc
    return _CACHE["nc"]


def kernel(x, w_off, b_off, w_def, b_def, gamma, beta, trace=False):
    x = np.asarray(x, np.float32)
    w_off = np.asarray(w_off, np.float32)
    b_off = np.asarray(b_off, np.float32)
    w_def = np.asarray(w_def, np.float32)
    gamma = np.asarray(gamma, np.float32)
    beta = np.asarray(beta, np.float32)
    # b_def cancels exactly in training-mode BN; accepted but unused.
    nc = _get_nc()
    in_maps = [_prep_core(x[b], w_off, b_off, w_def, gamma, beta)
               for b in range(B)]
    res = bass_utils.run_bass_kernel_spmd(
        nc, in_maps, core_ids=list(range(NCORES)), trace=trace)
    out = np.stack([res.results[b]["out"].reshape(C, H, W) for b in range(B)])
    if trace:
        kernel.last_exec_time_ns = res.exec_time_ns
        kernel.last_results = res
    return out



# revision 5
# speedup vs baseline: 2.0205x; 2.0205x over previous
"""Deformable conv block (offset conv -> bilinear sample -> conv -> BN -> ReLU)
on 8 Trainium2 NeuronCores, data-parallel over batch.

Self-contained: hardcodes all shapes. kernel(**inputs) takes full inputs,
shards batch across 8 cores, runs one Bass/Tile SPMD program, returns the
full [8, 64, 96, 96] float32 output.

Per-core device pipeline:
  1. offset conv (3x3, fp32r matmuls, channel-major)
  2. PE-transpose offsets to pixel-major [128, 72]
  3. per tap: bilinear weights + gather index q (magic-number floor; zero
     padding handled by a host-built padded sample table -> no OOB masks)
  4. per tap: build int16 gather indices in the HW 16-row-wrap layout via
     PE double-transpose, replicate to all 8 row groups by DMA
  5. per (chunk, tap): dma_gather 1KB descriptors from x2pad windows
     (one descriptor = 4 bilinear corners x 64 channels), DVE weighting
     with per-pixel broadcast APs, PE pair-transposes to channel-major,
     fp32r conv matmuls accumulating over taps in PSUM
  6. BN stats (ACT accum_out) + AllReduce [64,2] across the 8 cores,
     scale/bias fold (conv bias b_def cancels), ReLU, un-permuting DMA out
"""
import os
from contextlib import ExitStack

import numpy as np

import concourse.bass as bass
import concourse.tile as tile
from concourse import bacc, mybir, bass_utils

dt = mybir.dt
AOT = mybir.AluOpType
AFT = mybir.ActivationFunctionType

# problem shapes
B, C, H, W, K = 8, 64, 96, 96, 3
HW = H * W                # 9216
K2 = K * K                # 9
NCORES = 8
EPS = 1e-5

# padded sample-grid geometry: padded coord = image coord + PADM
PADM = 3                  # margin for floor(py) in [0, 100]
PW = W + 2 * PADM + 1     # 103 padded grid width
PR = H + 2 * PADM + 1     # 103 padded grid rows
NQ = PW * PR              # 10609 rows in x2pad
QCLAMP = float(W + 2 * PADM - 2)  # 100: floor clamp ceiling

NB = HW // 128            # 72 pixel-major block columns
CHUNK = 12                # block columns per main-loop chunk
NCH = NB // CHUNK         # 6 chunks
NPIX = CHUNK * 128        # 1536 pixels per chunk
MAGIC = 8388608.0         # 2^23

_CACHE = {}


def _build(nc, ncores=NCORES, use_collective=True):
    STOP = os.environ.get("KSTOP", "full")
    xpad = nc.dram_tensor("xpad", [C, 98 * 98], dt.float32r, kind="ExternalInput").ap()
    x2pad = nc.dram_tensor("x2pad", [NQ, 128], dt.float32, kind="ExternalInput").ap()
    woffT = nc.dram_tensor("woffT", [C, K2 * 18], dt.float32r, kind="ExternalInput").ap()
    wdefT = nc.dram_tensor("wdefT", [128, K2 * C], dt.float32r, kind="ExternalInput").ap()
    bpy = nc.dram_tensor("bpy", [128, K2 * NB], dt.float32, kind="ExternalInput").ap()
    bpx = nc.dram_tensor("bpx", [128, K2 * NB], dt.float32, kind="ExternalInput").ap()
    ident = nc.dram_tensor("ident", [128, 128], dt.float32, kind="ExternalInput").ap()
    idxh = nc.dram_tensor("idxh", [128, K2 * (HW // 16)], dt.int16,
                          kind="ExternalInput").ap()
    bnc = nc.dram_tensor("bnc", [C, 2], dt.float32, kind="ExternalInput").ap()
    out_d = nc.dram_tensor("out", [C, HW], dt.float32, kind="ExternalOutput").ap()

    with tile.TileContext(nc) as tc:
        with ExitStack() as ctx:
            cpool = ctx.enter_context(tc.tile_pool(name="const", bufs=1))
            ppool = ctx.enter_context(tc.tile_pool(name="persist", bufs=1))
            spool = ctx.enter_context(tc.tile_pool(name="small", bufs=2))
            gpool = ctx.enter_context(tc.tile_pool(name="gather", bufs=4))
            wpool = ctx.enter_context(tc.tile_pool(name="work", bufs=2))
            dpool = ctx.enter_context(tc.tile_pool(name="dram", bufs=1, space="DRAM"))
            ps_m = ctx.enter_context(tc.tile_pool(name="ps_m", bufs=2, space="PSUM"))
            ps_t = ctx.enter_context(tc.tile_pool(name="ps_t", bufs=1, space="PSUM"))
            ps_o = ctx.enter_context(tc.tile_pool(name="ps_o", bufs=1, space="PSUM"))

            # ---- load constants ----
            woffT_s = cpool.tile([C, K2 * 18], dt.float32r)
            nc.sync.dma_start(woffT_s[:], woffT)
            wdefT_s = cpool.tile([128, K2 * C], dt.float32r)
            nc.sync.dma_start(wdefT_s[:], wdefT)
            bpy_s = cpool.tile([128, K2 * NB], dt.float32)
            nc.sync.dma_start(bpy_s[:], bpy)
            bpx_s = cpool.tile([128, K2 * NB], dt.float32)
            nc.sync.dma_start(bpx_s[:], bpx)
            id_s = cpool.tile([128, 128], dt.float32)
            nc.sync.dma_start(id_s[:], ident)
            bnc_s = cpool.tile([C, 2], dt.float32)
            nc.sync.dma_start(bnc_s[:], bnc)

            # ---- 1+2. offset conv (streamed) + transpose to pixel-major ----
            # offT_s[p, 18*b + j] = off[j, 128*b + p]
            offT_s = ppool.tile([128, NB * 18], dt.float32)
            xpv = xpad.rearrange("c (h w) -> c h w", w=98)
            for cc in range(24):            # chunks of 4 rows = 384 px = 3 blocks
                xpc = wpool.tile([C, 6 * 98], dt.float32r, tag="xpc")
                nc.sync.dma_start(
                    xpc[:].rearrange("c (h w) -> c h w", w=98),
                    xpv[:, 4 * cc: 4 * cc + 6, :])
                xv = xpc[:].rearrange("c (h w) -> c h w", w=98)
                po = ps_m.tile([18, 384], dt.float32, tag="ps_misc")
                for k in range(K2):
                    ky, kx = k // K, k % K
                    rhs = xv[:, ky: ky + 4, kx: kx + 96]
                    nc.tensor.matmul(po[:], woffT_s[:, 18 * k: 18 * k + 18], rhs,
                                     start=(k == 0), stop=(k == K2 - 1))
                offc = wpool.tile([18, 384], dt.float32, tag="offc")
                nc.scalar.copy(offc[:], po[:])
                for cb in range(3):
                    pt = ps_m.tile([128, 18], dt.float32, tag="ps_misc")
                    nc.tensor.transpose(pt[:], offc[:, 128 * cb: 128 * cb + 128],
                                        id_s[0:18, 0:18])
                    c = 3 * cc + cb
                    nc.vector.tensor_copy(offT_s[:, 18 * c: 18 * c + 18], pt[:])
            offT_v = offT_s[:].rearrange("p (b j) -> p b j", j=18)
            if STOP == "off":
                nc.sync.dma_start(out_d[:, 0:NB * 18].rearrange("c (a b) -> c a b", a=2),
                                  offT_s[:].rearrange("p (a b) -> p a b", a=2)[0:64])
                return

            # ---- 3+4. per tap: weights w4 + int16 idx (16-wrap, replicated) ----
            w4_all = ppool.tile([128, K2 * NB * 4], dt.float32)
            w4v_all = w4_all[:].rearrange("p (k b j u) -> p k b j u", k=K2, j=4, u=1)
            idx_all = ppool.tile([128, K2 * (HW // 16)], dt.int16)
            for k in range(K2):
                py = spool.tile([128, NB], dt.float32, tag="py")
                nc.vector.tensor_tensor(py[:], offT_v[:, :, 2 * k],
                                        bpy_s[:, NB * k: NB * k + NB], AOT.add)
                px = spool.tile([128, NB], dt.float32, tag="px")
                nc.vector.tensor_tensor(px[:], offT_v[:, :, 2 * k + 1],
                                        bpx_s[:, NB * k: NB * k + NB], AOT.add)
                ry = spool.tile([128, NB], dt.float32, tag="ry")
                nc.vector.tensor_scalar(ry[:], py[:], MAGIC - 0.5, None, AOT.add)
                fy = spool.tile([128, NB], dt.float32, tag="fy")
                nc.vector.tensor_scalar(fy[:], ry[:], MAGIC, None, AOT.subtract)
                rx = spool.tile([128, NB], dt.float32, tag="rx")
                nc.vector.tensor_scalar(rx[:], px[:], MAGIC - 0.5, None, AOT.add)
                fx = spool.tile([128, NB], dt.float32, tag="fx")
                nc.vector.tensor_scalar(fx[:], rx[:], MAGIC, None, AOT.subtract)
                ly = spool.tile([128, NB], dt.float32, tag="ly")
                nc.vector.tensor_tensor(ly[:], py[:], fy[:], AOT.subtract)
                lx = spool.tile([128, NB], dt.float32, tag="lx")
                nc.vector.tensor_tensor(lx[:], px[:], fx[:], AOT.subtract)
                wy0 = spool.tile([128, NB], dt.float32, tag="wy0")
                nc.vector.tensor_scalar(wy0[:], ly[:], -1.0, 1.0, AOT.mult, AOT.add)
                wx0 = spool.tile([128, NB], dt.float32, tag="wx0")
                nc.vector.tensor_scalar(wx0[:], lx[:], -1.0, 1.0, AOT.mult, AOT.add)
                # corner blocks in x2pad window order: [c00, c10, c01, c11]
                w4v = w4v_all[:, k]
                nc.vector.tensor_tensor(w4v[:, :, 0, 0], wy0[:], wx0[:], AOT.mult)
                nc.vector.tensor_tensor(w4v[:, :, 1, 0], ly[:], wx0[:], AOT.mult)
                nc.vector.tensor_tensor(w4v[:, :, 2, 0], wy0[:], lx[:], AOT.mult)
                nc.vector.tensor_tensor(w4v[:, :, 3, 0], ly[:], lx[:], AOT.mult)
                qy = spool.tile([128, NB], dt.float32, tag="qy")
                nc.vector.tensor_scalar(qy[:], fy[:], 0.0, QCLAMP, AOT.max, AOT.min)
                qx = spool.tile([128, NB], dt.float32, tag="qx")
                nc.vector.tensor_scalar(qx[:], fx[:], 0.0, QCLAMP, AOT.max, AOT.min)
                qf = spool.tile([128, NB], dt.float32, tag="qf")
                nc.vector.scalar_tensor_tensor(qf[:], qy[:], float(PW), qx[:],
                                               AOT.mult, AOT.add)
                # idx construction: T1 then 8x T2 into 16-row staging
                t1p = ps_m.tile([NB, 128], dt.float32, tag="ps_misc")
                nc.tensor.transpose(t1p[:], qf[:], id_s[:])
                t1s = spool.tile([NB, 128], dt.float32, tag="t1s")
                nc.vector.tensor_copy(t1s[:], t1p[:])
                stv = idx_all[0:16, :].rearrange("p (k b r) -> p k b r", k=K2, r=8)
                for r in range(8):
                    t2p = ps_m.tile([16, NB], dt.float32, tag="ps_misc")
                    nc.tensor.transpose(t2p[:], t1s[:, 16 * r: 16 * r + 16],
                                        id_s[0:NB, 0:NB])
                    nc.any.tensor_copy(stv[:, k, :, r], t2p[:])
                # replicate rows 0..15 -> groups 1..7 for this tap
                for g in range(1, 8):
                    nc.sync.dma_start(
                        idx_all[16 * g: 16 * g + 16,
                                (HW // 16) * k: (HW // 16) * (k + 1)],
                        idx_all[0:16, (HW // 16) * k: (HW // 16) * (k + 1)])

            if STOP == "idx":
                nc.sync.dma_start(out_d[0:64, 0:K2 * NB * 4], w4_all[0:64, :])
                nc.sync.dma_start(
                    out_d[0:64, K2 * NB * 4: K2 * NB * 4 + K2 * HW // 16].bitcast(dt.int16)[:, 0:K2 * HW // 16],
                    idx_all[0:64, :])
                return

            if STOP in ("g1h", "hostidx"):
                nc.sync.dma_start(idx_all[:], idxh)

            # ---- 5. main loop: gather -> weight -> transpose -> conv ----
            DO_W = STOP not in ("g1", "g1h")
            DO_T = STOP in ("gt", "nm", "loop", "full")
            DO_MM = STOP in ("loop", "full")
            NCH_RUN = 1 if STOP in ("g1", "g1h", "gw", "gt", "nm") else NCH
            x2win = bass.AP(x2pad.tensor, 0, [[128, NQ - 1], [1, 256]])
            conv_s = ppool.tile([C, HW], dt.float32)
            sums = ppool.tile([C, 8], dt.float32)
            sqs = ppool.tile([C, 8], dt.float32)
            for ch in range(NCH_RUN):
                # 4 bank-aligned accumulation groups: e1 e2 | o1 o2
                po = ps_o.tile([C, 2048], dt.float32, tag="ps_out")
                for k in range(K2):
                    g_t = gpool.tile([128, CHUNK * 256], dt.float32, tag="g")
                    gview = g_t[:].rearrange("p (b e) -> p b e", e=256)
                    for g3 in range(3):
                        cbase = (HW // 16) * k + 96 * ch + 32 * g3
                        nc.gpsimd.dma_gather(
                            out_ap=gview[:, 4 * g3: 4 * g3 + 4, :],
                            in_ap=x2win,
                            idxs_ap=idx_all[:, cbase: cbase + 32],
                            num_idxs=512,
                            num_idxs_reg=512,
                            elem_size=256,
                            elem_step=128,
                            queue_num=(ch * K2 * 3 + k * 3 + g3) % 4,
                        )
                    if not DO_W:
                        nc.sync.dma_start(out_d[0:64, 0:CHUNK * 256], g_t[0:64, :])
                        return
                    gv = g_t[:].rearrange("p (b j c) -> p b j c", j=4, c=C)
                    s_t = wpool.tile([128, CHUNK * C], dt.float32, tag="s")
                    sv = s_t[:].rearrange("p (b c) -> p b c", c=C)
                    tmp = wpool.tile([128, CHUNK * C], dt.float32, tag="tmp")
                    tv = tmp[:].rearrange("p (b c) -> p b c", c=C)
                    for j in range(4):
                        wj = w4v_all[:, k, CHUNK * ch: CHUNK * ch + CHUNK, j, :]
                        a1, a2 = bass.broadcast_tensor_aps(gv[:, :, j, :], wj)
                        if j == 0:
                            nc.vector.tensor_tensor(sv, a1, a2, AOT.mult)
                        else:
                            nc.vector.tensor_tensor(tv, a1, a2, AOT.mult)
                            nc.vector.tensor_tensor(sv, sv, tv, AOT.add)
                    if not DO_T:
                        if k == K2 - 1:
                            nc.sync.dma_start(out_d[0:64, 0:CHUNK * C], s_t[0:64, :])
                            return
                        continue
                    # pair transposes -> channel-major sampled
                    pt = ps_t.tile([128, CHUNK * C], dt.float32, tag="ps_tr")
                    for bb in range(CHUNK // 2):
                        nc.tensor.transpose(pt[:, 128 * bb: 128 * bb + 128],
                                            s_t[:, 128 * bb: 128 * bb + 128],
                                            id_s[:])
                    samp = wpool.tile([128, CHUNK * C], dt.float32r, tag="samp")
                    nc.scalar.copy(samp[:], pt[:])
                    if not DO_MM:
                        if k == K2 - 1:
                            nc.sync.dma_start(out_d[0:64, 0:CHUNK * C], samp[0:64, :].bitcast(dt.float32))
                            return
                        continue
                    # conv matmuls: accumulate over taps, parity-major out cols
                    st, sp = (k == 0), (k == K2 - 1)
                    lhe = wdefT_s[0:64, C * k: C * k + C]
                    lho = wdefT_s[64:128, C * k: C * k + C]
                    nc.tensor.matmul(po[:, 0:512], lhe, samp[0:64, 0:512],
                                     start=st, stop=sp)
                    nc.tensor.matmul(po[:, 512:768], lhe, samp[0:64, 512:768],
                                     start=st, stop=sp)
                    nc.tensor.matmul(po[:, 1024:1536], lho, samp[64:128, 0:512],
                                     start=st, stop=sp)
                    nc.tensor.matmul(po[:, 1536:1792], lho, samp[64:128, 512:768],
                                     start=st, stop=sp)
                # copy conv chunk to SBUF + per-chunk sum / sumsq
                cview = conv_s[:, NPIX * ch: NPIX * ch + NPIX].rearrange(
                    "c (h x) -> c h x", x=768)
                pov = po[:].rearrange("c (h x) -> c h x", x=1024)[:, :, 0:768]
                nc.scalar.activation(cview, pov, AFT.Copy,
                                     accum_out=sums[:, ch: ch + 1])
                scr = wpool.tile([C, NPIX], dt.float32, tag="scr", bufs=1)
                nc.scalar.activation(scr[:], cview, AFT.Square,
                                     accum_out=sqs[:, ch: ch + 1])

            if STOP == "loop":
                nc.sync.dma_start(out_d[:], conv_s[:])
                return

            # ---- 6. BN stats allreduce + normalize + relu + output ----
            st2 = ppool.tile([C, 2], dt.float32)
            nc.vector.tensor_reduce(st2[:, 0:1], sums[:, 0:NCH],
                                    mybir.AxisListType.X, AOT.add)
            nc.vector.tensor_reduce(st2[:, 1:2], sqs[:, 0:NCH],
                                    mybir.AxisListType.X, AOT.add)
            bi = dpool.tile([C, 2], dt.float32)
            bo = dpool.tile([C, 2], dt.float32)
            nc.gpsimd.dma_start(bi[:], st2[:])
            if use_collective:
                nc.gpsimd.collective_compute(
                    "AllReduce", AOT.add,
                    replica_groups=[list(range(ncores))],
                    ins=[bi.opt()], outs=[bo.opt()])
            else:
                nc.gpsimd.dma_start(bo[:], bi[:])
            ast = ppool.tile([C, 2], dt.float32)
            nc.gpsimd.dma_start(ast[:], bo[:])

            inv_n = 1.0 / float(ncores * HW)
            mean = ppool.tile([C, 1], dt.float32)
            nc.vector.tensor_scalar(mean[:], ast[:, 0:1], inv_n, None, AOT.mult)
            msq = ppool.tile([C, 1], dt.float32)
            nc.vector.tensor_scalar(msq[:], ast[:, 1:2], inv_n, None, AOT.mult)
            m2 = ppool.tile([C, 1], dt.float32)
            nc.vector.tensor_tensor(m2[:], mean[:], mean[:], AOT.mult)
            var = ppool.tile([C, 1], dt.float32)
            nc.vector.tensor_tensor(var[:], msq[:], m2[:], AOT.subtract)
            vare = ppool.tile([C, 1], dt.float32)
            nc.vector.tensor_scalar(vare[:], var[:], EPS, None, AOT.add)
            sd = ppool.tile([C, 1], dt.float32)
            nc.scalar.activation(sd[:], vare[:], AFT.Sqrt)
            inv = ppool.tile([C, 1], dt.float32)
            nc.vector.reciprocal(inv[:], sd[:])
            scl = ppool.tile([C, 1], dt.float32)
            nc.vector.tensor_tensor(scl[:], bnc_s[:, 0:1], inv[:], AOT.mult)
            mt = ppool.tile([C, 1], dt.float32)
            nc.vector.tensor_tensor(mt[:], mean[:], scl[:], AOT.mult)
            bia = ppool.tile([C, 1], dt.float32)
            nc.vector.tensor_tensor(bia[:], bnc_s[:, 1:2], mt[:], AOT.subtract)

            ov = out_d.rearrange("c (n q) -> c n q", q=128)
            for ch in range(NCH):
                on = wpool.tile([C, NPIX], dt.float32, tag="on")
                nc.scalar.activation(on[:], conv_s[:, NPIX * ch: NPIX * ch + NPIX],
                                     AFT.Relu, bias=bia[:], scale=scl[:])
                onv = on[:].rearrange("c (n q) -> c n q", q=128)
                # even b -> pixel cols 1536ch + 256B + p ; odd -> +128
                nc.sync.dma_start(ov[:, 12 * ch: 12 * ch + 12: 2, :], onv[:, 0:6, :])
                nc.sync.dma_start(ov[:, 12 * ch + 1: 12 * ch + 12: 2, :], onv[:, 6:12, :])


def _prep_core(xb, w_off, b_off, w_def, gamma, beta):
    """Host-side input prep for one batch item. xb: [64, 96, 96] f32."""
    ins = {}
    # xpad: zero-pad by 1 for the 3x3 offset conv
    xp = np.zeros((C, 98, 98), np.float32)
    xp[:, 1:97, 1:97] = xb
    ins["xpad"] = xp.reshape(C, 98 * 98)
    # x2pad: padded HWC grid, rows [xz[r,s,:], xz[r+1,s,:]]
    xz = np.zeros((PR + 1, PW, C), np.float32)
    xz[PADM:PADM + H, PADM:PADM + W] = xb.transpose(1, 2, 0)
    xzf = xz.reshape((PR + 1) * PW, C)
    ins["x2pad"] = np.concatenate([xzf[0:NQ], xzf[PW:NQ + PW]], axis=1)
    # weight rearrangements
    wofft = np.zeros((C, K2 * 18), np.float32)
    for k in range(K2):
        wofft[:, 18 * k:18 * k + 18] = w_off[:, :, k // K, k % K].T
    ins["woffT"] = wofft
    wdeft = np.zeros((128, K2 * C), np.float32)
    for k in range(K2):
        blk = w_def[:, :, k // K, k % K].T  # [cin, cout]
        wdeft[0:64, C * k:C * k + C] = blk
        wdeft[64:128, C * k:C * k + C] = blk
    ins["wdefT"] = wdeft
    # base grids (pixel-major [128, 72] per tap), fold b_off and pad margin
    pixi = np.arange(HW, dtype=np.int64)
    ygrid = (pixi // W).astype(np.float32)
    xgrid = (pixi % W).astype(np.float32)
    ypm = ygrid.reshape(NB, 128).T    # [p, b] pixel-major
    xpm = xgrid.reshape(NB, 128).T
    bpy = np.zeros((128, K2 * NB), np.float32)
    bpx = np.zeros((128, K2 * NB), np.float32)
    for k in range(K2):
        ky, kx = k // K - 1, k % K - 1
        bpy[:, NB * k:NB * k + NB] = ypm + (ky + PADM + b_off[2 * k])
        bpx[:, NB * k:NB * k + NB] = xpm + (kx + PADM + b_off[2 * k + 1])
    ins["bpy"] = bpy
    ins["bpx"] = bpx
    ins["ident"] = np.eye(128, dtype=np.float32)
    ins["idxh"] = np.full((128, K2 * (HW // 16)), 5000, dtype=np.int16)
    ins["bnc"] = np.stack([gamma, beta], axis=1).astype(np.float32)
    return ins


def _get_nc():
    if "nc" not in _CACHE:
        nc = bacc.Bacc("TRN2", target_bir_lowering=False, debug=False,
                       num_devices=NCORES, num_swdge_queues=4)
        _build(nc)
        nc.compile()
        _CACHE["nc"] = nc
    return _CACHE["nc"]


def kernel(x, w_off, b_off, w_def, b_def, gamma, beta, trace=False):
    x = np.asarray(x, np.float32)
    w_off = np.asarray(w_off, np.float32)
    b_off = np.asarray(b_off, np.float32)
    w_def = np.asarray(w_def, np.float32)
    gamma = np.asarray(gamma, np.float32)
    beta = np.asarray(beta, np.float32)
    # b_def cancels exactly in training-mode BN; accepted but unused.
    nc = _get_nc()
    in_maps = [_prep_core(x[b], w_off, b_off, w_def, gamma, beta)
               for b in range(B)]
    res = bass_utils.run_bass_kernel_spmd(
        nc, in_maps, core_ids=list(range(NCORES)), trace=trace)
    out = np.stack([res.results[b]["out"].reshape(C, H, W) for b in range(B)])
    if trace:
        kernel.last_exec_time_ns = res.exec_time_ns
        kernel.last_results = res
    return out



# revision 6
# speedup vs baseline: 2.6967x; 1.3346x over previous
"""Deformable conv block (offset conv -> bilinear sample -> conv -> BN -> ReLU)
on 8 Trainium2 NeuronCores, data-parallel over batch.

Self-contained: hardcodes all shapes. kernel(**inputs) takes full inputs,
shards batch across 8 cores, runs one Bass/Tile SPMD program, returns the
full [8, 64, 96, 96] float32 output.

Per-core device pipeline:
  1. offset conv (3x3, fp32r matmuls, vertical 2-tap packing over 128
     contraction rows)
  2. PE-transpose offsets to pixel-major [128, 72]
  3. per tap: bilinear weights + gather index q (magic-number floor; zero
     padding handled by a host-built padded sample table -> no OOB masks)
  4. per tap: build int16 gather indices in the HW 16-row-wrap layout via
     PE double-transpose, replicate to all 8 row groups by DMA
  5. per (chunk, tap-pair): dma_gather 512B bf16 descriptors from x2pad
     windows (one descriptor = 4 bilinear corners x 64 channels), 4 SWDGE
     queues round-robin for parallel Q7 descriptor generation, DVE
     weighting with per-pixel broadcast APs into 2-tap-interleaved tiles,
     bf16 PE transposes to channel-major, one 128-deep matmul per tap pair
     accumulating over pairs in PSUM (natural pixel order)
  6. BN stats (ACT accum_out) + AllReduce [64,2] across the 8 cores,
     scale/bias fold (conv bias b_def cancels), ReLU, contiguous DMA out
"""
import os
from contextlib import ExitStack

import numpy as np
import ml_dtypes

import concourse.bass as bass
import concourse.tile as tile
from concourse import bacc, mybir, bass_utils

dt = mybir.dt
AOT = mybir.AluOpType
AFT = mybir.ActivationFunctionType

# problem shapes
B, C, H, W, K = 8, 64, 96, 96, 3
HW = H * W                # 9216
K2 = K * K                # 9
NPAIR = 5                 # ceil(9/2) tap pairs for 128-deep conv matmuls
NCORES = 8
EPS = 1e-5

# padded sample-grid geometry: padded coord = image coord + PADM
PADM = 3                  # margin for floor(py) in [0, 100]
PW = W + 2 * PADM + 1     # 103 padded grid width
PR = H + 2 * PADM + 1     # 103 padded grid rows
NQ = PW * PR              # 10609 rows in x2pad
QCLAMP = float(W + 2 * PADM - 2)  # 100: floor clamp ceiling

NB = HW // 128            # 72 pixel-major block columns
CHUNK = 12                # block columns per main-loop chunk
NCH = NB // CHUNK         # 6 chunks
NPIX = CHUNK * 128        # 1536 pixels per chunk
MAGIC = 8388608.0         # 2^23

_CACHE = {}


def _build(nc, ncores=NCORES, use_collective=True):
    STOP = os.environ.get("KSTOP", "full")
    xpad = nc.dram_tensor("xpad", [C, 99 * 98], dt.float32r, kind="ExternalInput").ap()
    x2pad = nc.dram_tensor("x2pad", [NQ, 128], dt.bfloat16, kind="ExternalInput").ap()
    woffT = nc.dram_tensor("woffT", [128, 6 * 18], dt.float32r, kind="ExternalInput").ap()
    wdefT = nc.dram_tensor("wdefT", [128, NPAIR * C], dt.bfloat16, kind="ExternalInput").ap()
    bpy = nc.dram_tensor("bpy", [128, K2 * NB], dt.float32, kind="ExternalInput").ap()
    bpx = nc.dram_tensor("bpx", [128, K2 * NB], dt.float32, kind="ExternalInput").ap()
    ident = nc.dram_tensor("ident", [128, 128], dt.float32, kind="ExternalInput").ap()
    identb = nc.dram_tensor("identb", [128, 128], dt.bfloat16, kind="ExternalInput").ap()
    bnc = nc.dram_tensor("bnc", [C, 2], dt.float32, kind="ExternalInput").ap()
    out_d = nc.dram_tensor("out", [C, HW], dt.float32, kind="ExternalOutput").ap()

    with tile.TileContext(nc) as tc:
        with ExitStack() as ctx:
            cpool = ctx.enter_context(tc.tile_pool(name="const", bufs=1))
            ppool = ctx.enter_context(tc.tile_pool(name="persist", bufs=1))
            spool = ctx.enter_context(tc.tile_pool(name="small", bufs=2))
            gpool = ctx.enter_context(tc.tile_pool(name="gather", bufs=4))
            wpool = ctx.enter_context(tc.tile_pool(name="work", bufs=2))
            dpool = ctx.enter_context(tc.tile_pool(name="dram", bufs=1, space="DRAM"))
            ps_m = ctx.enter_context(tc.tile_pool(name="ps_m", bufs=2, space="PSUM"))
            ps_t = ctx.enter_context(tc.tile_pool(name="ps_t", bufs=2, space="PSUM"))
            ps_o = ctx.enter_context(tc.tile_pool(name="ps_o", bufs=1, space="PSUM"))

            # ---- load constants ----
            woffT_s = cpool.tile([128, 6 * 18], dt.float32r)
            nc.sync.dma_start(woffT_s[:], woffT)
            wdefT_s = cpool.tile([128, NPAIR * C], dt.bfloat16)
            nc.sync.dma_start(wdefT_s[:], wdefT)
            bpy_s = cpool.tile([128, K2 * NB], dt.float32)
            nc.sync.dma_start(bpy_s[:], bpy)
            bpx_s = cpool.tile([128, K2 * NB], dt.float32)
            nc.sync.dma_start(bpx_s[:], bpx)
            id_s = cpool.tile([128, 128], dt.float32)
            nc.sync.dma_start(id_s[:], ident)
            idb_s = cpool.tile([128, 128], dt.bfloat16)
            nc.sync.dma_start(idb_s[:], identb)
            bnc_s = cpool.tile([C, 2], dt.float32)
            nc.sync.dma_start(bnc_s[:], bnc)

            # ---- 1+2. offset conv (streamed, ky 0/1 packed) + transpose ----
            # offT_s[p, 18*b + j] = off[j, 128*b + p]
            offT_s = ppool.tile([128, NB * 18], dt.float32)
            xpv = xpad.rearrange("c (h w) -> c h w", w=98)
            for cc in range(24):            # chunks of 4 rows = 384 px = 3 blocks
                xpc = wpool.tile([128, 6 * 98], dt.float32r, tag="xpc")
                xvc = xpc[:].rearrange("c (h w) -> c h w", w=98)
                nc.sync.dma_start(xvc[0:C], xpv[:, 4 * cc: 4 * cc + 6, :])
                nc.sync.dma_start(xvc[C:128], xpv[:, 4 * cc + 1: 4 * cc + 7, :])
                po = ps_m.tile([18, 384], dt.float32, tag="ps_misc")
                for kx in range(K):
                    # taps (ky=0,kx) + (ky=1,kx) fused over 128 rows
                    nc.tensor.matmul(po[:], woffT_s[:, 18 * kx: 18 * kx + 18],
                                     xvc[:, 0:4, kx: kx + 96],
                                     start=(kx == 0), stop=False)
                for kx in range(K):
                    # single tap (ky=2,kx) over the top 64 rows
                    nc.tensor.matmul(po[:], woffT_s[0:C, 18 * (3 + kx): 18 * (3 + kx) + 18],
                                     xvc[0:C, 2:6, kx: kx + 96],
                                     start=False, stop=(kx == K - 1))
                offc = wpool.tile([18, 384], dt.float32, tag="offc")
                nc.scalar.copy(offc[:], po[:])
                for cb in range(3):
                    pt = ps_m.tile([128, 18], dt.float32, tag="ps_misc")
                    nc.tensor.transpose(pt[:], offc[:, 128 * cb: 128 * cb + 128],
                                        id_s[0:18, 0:18])
                    c = 3 * cc + cb
                    nc.vector.tensor_copy(offT_s[:, 18 * c: 18 * c + 18], pt[:])
            offT_v = offT_s[:].rearrange("p (b j) -> p b j", j=18)
            if STOP == "off":
                nc.sync.dma_start(out_d[:, 0:NB * 18].rearrange("c (a b) -> c a b", a=2),
                                  offT_s[:].rearrange("p (a b) -> p a b", a=2)[0:64])
                return

            # ---- 3+4. per tap: weights w4 + int16 idx (16-wrap, replicated) ----
            w4_all = ppool.tile([128, K2 * NB * 4], dt.bfloat16)
            w4v_all = w4_all[:].rearrange("p (k b j u) -> p k b j u", k=K2, j=4, u=1)
            idx_all = ppool.tile([128, K2 * (HW // 16)], dt.int16)
            for k in range(K2):
                py = spool.tile([128, NB], dt.float32, tag="py")
                nc.vector.tensor_tensor(py[:], offT_v[:, :, 2 * k],
                                        bpy_s[:, NB * k: NB * k + NB], AOT.add)
                px = spool.tile([128, NB], dt.float32, tag="px")
                nc.vector.tensor_tensor(px[:], offT_v[:, :, 2 * k + 1],
                                        bpx_s[:, NB * k: NB * k + NB], AOT.add)
                ry = spool.tile([128, NB], dt.float32, tag="ry")
                nc.vector.tensor_scalar(ry[:], py[:], MAGIC - 0.5, None, AOT.add)
                fy = spool.tile([128, NB], dt.float32, tag="fy")
                nc.vector.tensor_scalar(fy[:], ry[:], MAGIC, None, AOT.subtract)
                rx = spool.tile([128, NB], dt.float32, tag="rx")
                nc.vector.tensor_scalar(rx[:], px[:], MAGIC - 0.5, None, AOT.add)
                fx = spool.tile([128, NB], dt.float32, tag="fx")
                nc.vector.tensor_scalar(fx[:], rx[:], MAGIC, None, AOT.subtract)
                ly = spool.tile([128, NB], dt.float32, tag="ly")
                nc.vector.tensor_tensor(ly[:], py[:], fy[:], AOT.subtract)
                lx = spool.tile([128, NB], dt.float32, tag="lx")
                nc.vector.tensor_tensor(lx[:], px[:], fx[:], AOT.subtract)
                wy0 = spool.tile([128, NB], dt.float32, tag="wy0")
                nc.vector.tensor_scalar(wy0[:], ly[:], -1.0, 1.0, AOT.mult, AOT.add)
                wx0 = spool.tile([128, NB], dt.float32, tag="wx0")
                nc.vector.tensor_scalar(wx0[:], lx[:], -1.0, 1.0, AOT.mult, AOT.add)
                # corner blocks in x2pad window order: [c00, c10, c01, c11]
                w4v = w4v_all[:, k]
                nc.vector.tensor_tensor(w4v[:, :, 0, 0], wy0[:], wx0[:], AOT.mult)
                nc.vector.tensor_tensor(w4v[:, :, 1, 0], ly[:], wx0[:], AOT.mult)
                nc.vector.tensor_tensor(w4v[:, :, 2, 0], wy0[:], lx[:], AOT.mult)
                nc.vector.tensor_tensor(w4v[:, :, 3, 0], ly[:], lx[:], AOT.mult)
                qy = spool.tile([128, NB], dt.float32, tag="qy")
                nc.vector.tensor_scalar(qy[:], fy[:], 0.0, QCLAMP, AOT.max, AOT.min)
                qx = spool.tile([128, NB], dt.float32, tag="qx")
                nc.vector.tensor_scalar(qx[:], fx[:], 0.0, QCLAMP, AOT.max, AOT.min)
                qf = spool.tile([128, NB], dt.float32, tag="qf")
                nc.vector.scalar_tensor_tensor(qf[:], qy[:], float(PW), qx[:],
                                               AOT.mult, AOT.add)
                # idx construction: T1 then 8x T2 into 16-row staging
                t1p = ps_m.tile([NB, 128], dt.float32, tag="ps_misc")
                nc.tensor.transpose(t1p[:], qf[:], id_s[:])
                t1s = spool.tile([NB, 128], dt.float32, tag="t1s")
                nc.vector.tensor_copy(t1s[:], t1p[:])
                stv = idx_all[0:16, :].rearrange("p (k b r) -> p k b r", k=K2, r=8)
                for r in range(8):
                    t2p = ps_m.tile([16, NB], dt.float32, tag="ps_misc")
                    nc.tensor.transpose(t2p[:], t1s[:, 16 * r: 16 * r + 16],
                                        id_s[0:NB, 0:NB])
                    nc.any.tensor_copy(stv[:, k, :, r], t2p[:])
                # replicate rows 0..15 -> groups 1..7 for this tap
                for g in range(1, 8):
                    nc.sync.dma_start(
                        idx_all[16 * g: 16 * g + 16,
                                (HW // 16) * k: (HW // 16) * (k + 1)],
                        idx_all[0:16, (HW // 16) * k: (HW // 16) * (k + 1)])

            if STOP == "idx":
                nc.sync.dma_start(
                    out_d[0:64, 0:K2 * NB * 4].bitcast(dt.bfloat16)[:, 0:K2 * NB * 4],
                    w4_all[0:64, :])
                return

            # ---- 5. main loop: gather -> weight -> transpose -> conv ----
            DO_W = STOP not in ("g1",)
            DO_T = STOP in ("gt", "loop", "full")
            DO_MM = STOP in ("loop", "full")
            NCH_RUN = 1 if STOP in ("g1", "gw", "gt") else NCH
            x2win = bass.AP(x2pad.tensor, 0, [[128, NQ - 1], [1, 256]])
            conv_s = ppool.tile([C, HW], dt.float32)
            sums = ppool.tile([C, 8], dt.float32)
            sqs = ppool.tile([C, 8], dt.float32)
            gq = 0
            for ch in range(NCH_RUN):
                po = ps_o.tile([C, NPIX], dt.float32, tag="ps_out")
                for pr in range(NPAIR):
                    s2 = wpool.tile([128, CHUNK * 128], dt.bfloat16, tag="s2")
                    s2v = s2[:].rearrange("p (b h c) -> p b h c", h=2, c=C)
                    for h in range(2):
                        k = 2 * pr + h
                        if k == K2:
                            # dummy tap: stale bf16 data x zero lhsT rows
                            continue
                        g_t = gpool.tile([128, CHUNK * 256], dt.bfloat16, tag="g")
                        gview = g_t[:].rearrange("p (b e) -> p b e", e=256)
                        for g3 in range(3):
                            cbase = (HW // 16) * k + 96 * ch + 32 * g3
                            nc.gpsimd.dma_gather(
                                out_ap=gview[:, 4 * g3: 4 * g3 + 4, :],
                                in_ap=x2win,
                                idxs_ap=idx_all[:, cbase: cbase + 32],
                                num_idxs=512,
                                num_idxs_reg=512,
                                elem_size=256,
                                elem_step=128,
                                queue_num=gq % 4,
                            )
                            gq += 1
                        if not DO_W:
                            nc.sync.dma_start(
                                out_d[0:64, 0:CHUNK * 128].bitcast(dt.bfloat16)[:, 0:CHUNK * 256],
                                g_t[0:64, :])
                            return
                        gv = g_t[:].rearrange("p (b j c) -> p b j c", j=4, c=C)
                        sv = s2v[:, :, h, :]
                        tmp = wpool.tile([128, CHUNK * C], dt.bfloat16, tag="tmp")
                        tv = tmp[:].rearrange("p (b c) -> p b c", c=C)
                        for j in range(4):
                            wj = w4v_all[:, k, CHUNK * ch: CHUNK * ch + CHUNK, j, :]
                            a1, a2 = bass.broadcast_tensor_aps(gv[:, :, j, :], wj)
                            if j == 0:
                                nc.vector.tensor_tensor(sv, a1, a2, AOT.mult)
                            else:
                                nc.vector.tensor_tensor(tv, a1, a2, AOT.mult)
                                nc.vector.tensor_tensor(sv, sv, tv, AOT.add)
                    if not DO_T:
                        if pr == NPAIR - 1:
                            nc.sync.dma_start(
                                out_d[0:64, 0:CHUNK * 64].bitcast(dt.bfloat16)[:, 0:CHUNK * 128],
                                s2[0:64, :])
                            return
                        continue
                    # transposes -> channel-major sampled (3 blocks per group)
                    samp = wpool.tile([128, CHUNK * 128], dt.bfloat16, tag="samp")
                    for grp in range(CHUNK // 3):
                        pt = ps_t.tile([128, 384], dt.bfloat16, tag="ps_tr")
                        for bb in range(3):
                            col = 384 * grp + 128 * bb
                            nc.tensor.transpose(pt[:, 128 * bb: 128 * bb + 128],
                                                s2[:, col: col + 128], idb_s[:])
                        nc.scalar.copy(samp[:, 384 * grp: 384 * grp + 384], pt[:])
                    if not DO_MM:
                        if pr == NPAIR - 1:
                            nc.sync.dma_start(
                                out_d[0:64, 0:CHUNK * 64].bitcast(dt.bfloat16)[:, 0:CHUNK * 128],
                                samp[0:64, :])
                            return
                        continue
                    # one 128-deep matmul per tap pair, natural pixel order
                    st, sp = (pr == 0), (pr == NPAIR - 1)
                    lh = wdefT_s[:, C * pr: C * pr + C]
                    for j3 in range(3):
                        nc.tensor.matmul(po[:, 512 * j3: 512 * j3 + 512], lh,
                                         samp[:, 512 * j3: 512 * j3 + 512],
                                         start=st, stop=sp)
                # copy conv chunk to SBUF + per-chunk sum / sumsq
                cview = conv_s[:, NPIX * ch: NPIX * ch + NPIX]
                nc.scalar.activation(cview, po[:], AFT.Copy,
                                     accum_out=sums[:, ch: ch + 1])
                scr = wpool.tile([C, NPIX], dt.float32, tag="scr", bufs=1)
                nc.scalar.activation(scr[:], cview, AFT.Square,
                                     accum_out=sqs[:, ch: ch + 1])

            if STOP == "loop":
                nc.sync.dma_start(out_d[:], conv_s[:])
                return

            # ---- 6. BN stats allreduce + normalize + relu + output ----
            st2 = ppool.tile([C, 2], dt.float32)
            nc.vector.tensor_reduce(st2[:, 0:1], sums[:, 0:NCH],
                                    mybir.AxisListType.X, AOT.add)
            nc.vector.tensor_reduce(st2[:, 1:2], sqs[:, 0:NCH],
                                    mybir.AxisListType.X, AOT.add)
            bi = dpool.tile([C, 2], dt.float32)
            bo = dpool.tile([C, 2], dt.float32)
            nc.gpsimd.dma_start(bi[:], st2[:])
            if use_collective:
                nc.gpsimd.collective_compute(
                    "AllReduce", AOT.add,
                    replica_groups=[list(range(ncores))],
                    ins=[bi.opt()], outs=[bo.opt()])
            else:
                nc.gpsimd.dma_start(bo[:], bi[:])
            ast = ppool.tile([C, 2], dt.float32)
            nc.gpsimd.dma_start(ast[:], bo[:])

            inv_n = 1.0 / float(ncores * HW)
            mean = ppool.tile([C, 1], dt.float32)
            nc.vector.tensor_scalar(mean[:], ast[:, 0:1], inv_n, None, AOT.mult)
            msq = ppool.tile([C, 1], dt.float32)
            nc.vector.tensor_scalar(msq[:], ast[:, 1:2], inv_n, None, AOT.mult)
            m2 = ppool.tile([C, 1], dt.float32)
            nc.vector.tensor_tensor(m2[:], mean[:], mean[:], AOT.mult)
            var = ppool.tile([C, 1], dt.float32)
            nc.vector.tensor_tensor(var[:], msq[:], m2[:], AOT.subtract)
            vare = ppool.tile([C, 1], dt.float32)
            nc.vector.tensor_scalar(vare[:], var[:], EPS, None, AOT.add)
            sd = ppool.tile([C, 1], dt.float32)
            nc.scalar.activation(sd[:], vare[:], AFT.Sqrt)
            inv = ppool.tile([C, 1], dt.float32)
            nc.vector.reciprocal(inv[:], sd[:])
            scl = ppool.tile([C, 1], dt.float32)
            nc.vector.tensor_tensor(scl[:], bnc_s[:, 0:1], inv[:], AOT.mult)
            mt = ppool.tile([C, 1], dt.float32)
            nc.vector.tensor_tensor(mt[:], mean[:], scl[:], AOT.mult)
            bia = ppool.tile([C, 1], dt.float32)
            nc.vector.tensor_tensor(bia[:], bnc_s[:, 1:2], mt[:], AOT.subtract)

            for ch in range(NCH):
                on = wpool.tile([C, NPIX], dt.float32, tag="on")
                nc.scalar.activation(on[:], conv_s[:, NPIX * ch: NPIX * ch + NPIX],
                                     AFT.Relu, bias=bia[:], scale=scl[:])
                nc.sync.dma_start(out_d[:, NPIX * ch: NPIX * ch + NPIX], on[:])


def _prep_core(xb, w_off, b_off, w_def, gamma, beta):
    """Host-side input prep for one batch item. xb: [64, 96, 96] f32."""
    ins = {}
    # xpad: zero-pad for the 3x3 offset conv; 99 rows so the ky-shifted
    # bottom half's 6-row DMA stays in range for the last chunk
    xp = np.zeros((C, 99, 98), np.float32)
    xp[:, 1:97, 1:97] = xb
    ins["xpad"] = xp.reshape(C, 99 * 98)
    # x2pad: padded HWC grid (bf16), rows [xz[r,s,:], xz[r+1,s,:]]
    xz = np.zeros((PR + 1, PW, C), np.float32)
    xz[PADM:PADM + H, PADM:PADM + W] = xb.transpose(1, 2, 0)
    xzf = xz.reshape((PR + 1) * PW, C)
    x2 = np.concatenate([xzf[0:NQ], xzf[PW:NQ + PW]], axis=1)
    ins["x2pad"] = x2.astype(ml_dtypes.bfloat16)
    # offset conv weights: cols kx 0..2 = vertical pair (ky0+ky1), 3+kx = ky2
    wofft = np.zeros((128, 6 * 18), np.float32)
    for kx in range(K):
        wofft[0:C, 18 * kx:18 * kx + 18] = w_off[:, :, 0, kx].T
        wofft[C:128, 18 * kx:18 * kx + 18] = w_off[:, :, 1, kx].T
        wofft[0:C, 18 * (3 + kx):18 * (3 + kx) + 18] = w_off[:, :, 2, kx].T
    ins["woffT"] = wofft
    # deformable conv weights: tap pairs stacked over 128 contraction rows
    wdeft = np.zeros((128, NPAIR * C), np.float32)
    for pr in range(NPAIR):
        for h in range(2):
            k = 2 * pr + h
            if k < K2:
                wdeft[C * h:C * h + C, C * pr:C * pr + C] = \
                    w_def[:, :, k // K, k % K].T
    ins["wdefT"] = wdeft.astype(ml_dtypes.bfloat16)
    # base grids (pixel-major [128, 72] per tap), fold b_off and pad margin
    pixi = np.arange(HW, dtype=np.int64)
    ygrid = (pixi // W).astype(np.float32)
    xgrid = (pixi % W).astype(np.float32)
    ypm = ygrid.reshape(NB, 128).T    # [p, b] pixel-major
    xpm = xgrid.reshape(NB, 128).T
    bpy = np.zeros((128, K2 * NB), np.float32)
    bpx = np.zeros((128, K2 * NB), np.float32)
    for k in range(K2):
        ky, kx = k // K - 1, k % K - 1
        bpy[:, NB * k:NB * k + NB] = ypm + (ky + PADM + b_off[2 * k])
        bpx[:, NB * k:NB * k + NB] = xpm + (kx + PADM + b_off[2 * k + 1])
    ins["bpy"] = bpy
    ins["bpx"] = bpx
    ins["ident"] = np.eye(128, dtype=np.float32)
    ins["identb"] = np.eye(128, dtype=np.float32).astype(ml_dtypes.bfloat16)
    ins["bnc"] = np.stack([gamma, beta], axis=1).astype(np.float32)
    return ins


def _get_nc():
    if "nc" not in _CACHE:
        nc = bacc.Bacc("TRN2", target_bir_lowering=False, debug=False,
                       num_devices=NCORES, num_swdge_queues=4)
        _build(nc)
        nc.compile()
        _CACHE["nc"] = nc
    return _CACHE["nc"]


def kernel(x, w_off, b_off, w_def, b_def, gamma, beta, trace=False):
    x = np.asarray(x, np.float32)
    w_off = np.asarray(w_off, np.float32)
    b_off = np.asarray(b_off, np.float32)
    w_def = np.asarray(w_def, np.float32)
    gamma = np.asarray(gamma, np.float32)
    beta = np.asarray(beta, np.float32)
    # b_def cancels exactly in training-mode BN; accepted but unused.
    nc = _get_nc()
    in_maps = [_prep_core(x[b], w_off, b_off, w_def, gamma, beta)
               for b in range(B)]
    res = bass_utils.run_bass_kernel_spmd(
        nc, in_maps, core_ids=list(range(NCORES)), trace=trace)
    out = np.stack([res.results[b]["out"].reshape(C, H, W) for b in range(B)])
    if trace:
        kernel.last_exec_time_ns = res.exec_time_ns
        kernel.last_results = res
    return out


# revision 9
# speedup vs baseline: 3.0454x; 1.1293x over previous
"""Deformable conv block (offset conv -> bilinear sample -> conv -> BN -> ReLU)
on 8 Trainium2 NeuronCores, data-parallel over batch.

Self-contained: hardcodes all shapes. kernel(**inputs) takes full inputs,
shards batch across 8 cores, runs one Bass/Tile SPMD program, returns the
full [8, 64, 96, 96] float32 output.

Per-core device pipeline:
  1. offset conv (3x3, fp32r matmuls, vertical 2-tap packing over 128
     contraction rows)
  2. PE-transpose offsets to pixel-major [128, 72]
  3. per tap: bilinear weights + gather index q (magic-number floor; zero
     padding handled by a host-built padded sample table -> no OOB masks)
  4. per tap: build int16 gather indices in the HW 16-row-wrap layout via
     PE double-transpose, replicate to all 8 row groups by DMA
  5. per (chunk, tap-pair): dma_gather 512B bf16 descriptors from x2pad
     windows (one descriptor = 4 bilinear corners x 64 channels), 4 SWDGE
     queues round-robin for parallel Q7 descriptor generation, DVE
     weighting with per-pixel broadcast APs into 2-tap-interleaved tiles,
     bf16 PE transposes to channel-major, one 128-deep matmul per tap pair
     accumulating over pairs in PSUM (natural pixel order)
  6. BN stats (ACT accum_out) + AllReduce [64,2] across the 8 cores,
     scale/bias fold (conv bias b_def cancels), ReLU, contiguous DMA out
"""
import os
from contextlib import ExitStack

import numpy as np
import ml_dtypes

import concourse.bass as bass
import concourse.tile as tile
from concourse import bacc, mybir, bass_utils

dt = mybir.dt
AOT = mybir.AluOpType
AFT = mybir.ActivationFunctionType

# problem shapes
B, C, H, W, K = 8, 64, 96, 96, 3
HW = H * W                # 9216
K2 = K * K                # 9
NPAIR = 5                 # ceil(9/2) tap pairs for 128-deep conv matmuls
NCORES = 8
EPS = 1e-5

# padded sample-grid geometry: padded coord = image coord + PADM
PADM = 3                  # margin for floor(py) in [0, 100]
PW = W + 2 * PADM + 1     # 103 padded grid width
PR = H + 2 * PADM + 1     # 103 padded grid rows
NQ = PW * PR              # 10609 rows in x2pad
QCLAMP = float(W + 2 * PADM - 2)  # 100: floor clamp ceiling

NB = HW // 128            # 72 pixel-major block columns
CHUNK = 12                # block columns per main-loop chunk
NCH = NB // CHUNK         # 6 chunks
NPIX = CHUNK * 128        # 1536 pixels per chunk
MAGIC = 8388608.0         # 2^23

_CACHE = {}


def _build(nc, ncores=NCORES, use_collective=True):
    STOP = os.environ.get("KSTOP", "full")
    xpad = nc.dram_tensor("xpad", [C, 99 * 98], dt.float32r, kind="ExternalInput").ap()
    x2pad = nc.dram_tensor("x2pad", [NQ, 128], dt.bfloat16, kind="ExternalInput").ap()
    woffT = nc.dram_tensor("woffT", [128, 6 * 18], dt.float32r, kind="ExternalInput").ap()
    wdefT = nc.dram_tensor("wdefT", [128, NPAIR * C], dt.bfloat16, kind="ExternalInput").ap()
    bpy = nc.dram_tensor("bpy", [128, K2 * NB], dt.float32, kind="ExternalInput").ap()
    bpx = nc.dram_tensor("bpx", [128, K2 * NB], dt.float32, kind="ExternalInput").ap()
    ident = nc.dram_tensor("ident", [128, 128], dt.float32, kind="ExternalInput").ap()
    identb = nc.dram_tensor("identb", [128, 128], dt.bfloat16, kind="ExternalInput").ap()
    bnc = nc.dram_tensor("bnc", [C, 2], dt.float32, kind="ExternalInput").ap()
    out_d = nc.dram_tensor("out", [C, HW], dt.float32, kind="ExternalOutput").ap()

    with tile.TileContext(nc) as tc:
        with ExitStack() as ctx:
            cpool = ctx.enter_context(tc.tile_pool(name="const", bufs=1))
            ppool = ctx.enter_context(tc.tile_pool(name="persist", bufs=1))
            spool = ctx.enter_context(tc.tile_pool(name="small", bufs=2))
            gpool = ctx.enter_context(tc.tile_pool(name="gather", bufs=4))
            wpool = ctx.enter_context(tc.tile_pool(name="work", bufs=2))
            dpool = ctx.enter_context(tc.tile_pool(name="dram", bufs=1, space="DRAM"))
            ps_m = ctx.enter_context(tc.tile_pool(name="ps_m", bufs=2, space="PSUM"))
            ps_t = ctx.enter_context(tc.tile_pool(name="ps_t", bufs=2, space="PSUM"))
            ps_o = ctx.enter_context(tc.tile_pool(name="ps_o", bufs=1, space="PSUM"))

            # ---- load constants ----
            woffT_s = cpool.tile([128, 6 * 18], dt.float32r)
            nc.sync.dma_start(woffT_s[:], woffT)
            wdefT_s = cpool.tile([128, NPAIR * C], dt.bfloat16)
            nc.sync.dma_start(wdefT_s[:], wdefT)
            bpy_s = cpool.tile([128, K2 * NB], dt.float32)
            nc.sync.dma_start(bpy_s[:], bpy)
            bpx_s = cpool.tile([128, K2 * NB], dt.float32)
            nc.sync.dma_start(bpx_s[:], bpx)
            id_s = cpool.tile([128, 128], dt.float32)
            nc.sync.dma_start(id_s[:], ident)
            idb_s = cpool.tile([128, 128], dt.bfloat16)
            nc.sync.dma_start(idb_s[:], identb)
            bnc_s = cpool.tile([C, 2], dt.float32)
            nc.sync.dma_start(bnc_s[:], bnc)

            # ---- 1+2. offset conv (streamed, ky 0/1 packed) + transpose ----
            # offT_s[p, 18*b + j] = off[j, 128*b + p]
            offT_s = ppool.tile([128, NB * 18], dt.float32)
            xpv = xpad.rearrange("c (h w) -> c h w", w=98)
            for cc in range(24):            # chunks of 4 rows = 384 px = 3 blocks
                xpc = wpool.tile([128, 6 * 98], dt.float32r, tag="xpc")
                xvc = xpc[:].rearrange("c (h w) -> c h w", w=98)
                nc.sync.dma_start(xvc[0:C], xpv[:, 4 * cc: 4 * cc + 6, :])
                nc.sync.dma_start(xvc[C:128], xpv[:, 4 * cc + 1: 4 * cc + 7, :])
                po = ps_m.tile([18, 384], dt.float32, tag="ps_misc")
                for kx in range(K):
                    # taps (ky=0,kx) + (ky=1,kx) fused over 128 rows
                    nc.tensor.matmul(po[:], woffT_s[:, 18 * kx: 18 * kx + 18],
                                     xvc[:, 0:4, kx: kx + 96],
                                     start=(kx == 0), stop=False)
                for kx in range(K):
                    # single tap (ky=2,kx) over the top 64 rows
                    nc.tensor.matmul(po[:], woffT_s[0:C, 18 * (3 + kx): 18 * (3 + kx) + 18],
                                     xvc[0:C, 2:6, kx: kx + 96],
                                     start=False, stop=(kx == K - 1))
                offc = wpool.tile([18, 384], dt.float32, tag="offc")
                nc.scalar.copy(offc[:], po[:])
                for cb in range(3):
                    pt = ps_m.tile([128, 18], dt.float32, tag="ps_misc")
                    nc.tensor.transpose(pt[:], offc[:, 128 * cb: 128 * cb + 128],
                                        id_s[0:18, 0:18])
                    c = 3 * cc + cb
                    nc.vector.tensor_copy(offT_s[:, 18 * c: 18 * c + 18], pt[:])
            offT_v = offT_s[:].rearrange("p (b j) -> p b j", j=18)
            if STOP == "off":
                nc.sync.dma_start(out_d[:, 0:NB * 18].rearrange("c (a b) -> c a b", a=2),
                                  offT_s[:].rearrange("p (a b) -> p a b", a=2)[0:64])
                return

            # ---- 3+4. per tap: weights w4 + int16 idx (16-wrap, replicated) ----
            w4_all = ppool.tile([128, K2 * NB * 4], dt.bfloat16)
            w4v_all = w4_all[:].rearrange("p (k b j u) -> p k b j u", k=K2, j=4, u=1)
            idx_all = ppool.tile([128, K2 * (HW // 16)], dt.int16)
            for k in range(K2):
                py = spool.tile([128, NB], dt.float32, tag="py")
                nc.vector.tensor_tensor(py[:], offT_v[:, :, 2 * k],
                                        bpy_s[:, NB * k: NB * k + NB], AOT.add)
                px = spool.tile([128, NB], dt.float32, tag="px")
                nc.vector.tensor_tensor(px[:], offT_v[:, :, 2 * k + 1],
                                        bpx_s[:, NB * k: NB * k + NB], AOT.add)
                ry = spool.tile([128, NB], dt.float32, tag="ry")
                nc.vector.tensor_scalar(ry[:], py[:], MAGIC - 0.5, None, AOT.add)
                fy = spool.tile([128, NB], dt.float32, tag="fy")
                nc.vector.tensor_scalar(fy[:], ry[:], MAGIC, None, AOT.subtract)
                rx = spool.tile([128, NB], dt.float32, tag="rx")
                nc.vector.tensor_scalar(rx[:], px[:], MAGIC - 0.5, None, AOT.add)
                fx = spool.tile([128, NB], dt.float32, tag="fx")
                nc.vector.tensor_scalar(fx[:], rx[:], MAGIC, None, AOT.subtract)
                ly = spool.tile([128, NB], dt.float32, tag="ly")
                nc.vector.tensor_tensor(ly[:], py[:], fy[:], AOT.subtract)
                lx = spool.tile([128, NB], dt.float32, tag="lx")
                nc.vector.tensor_tensor(lx[:], px[:], fx[:], AOT.subtract)
                wy0 = spool.tile([128, NB], dt.float32, tag="wy0")
                nc.vector.tensor_scalar(wy0[:], ly[:], -1.0, 1.0, AOT.mult, AOT.add)
                wx0 = spool.tile([128, NB], dt.float32, tag="wx0")
                nc.vector.tensor_scalar(wx0[:], lx[:], -1.0, 1.0, AOT.mult, AOT.add)
                # corner blocks in x2pad window order: [c00, c10, c01, c11]
                w4v = w4v_all[:, k]
                nc.vector.tensor_tensor(w4v[:, :, 0, 0], wy0[:], wx0[:], AOT.mult)
                nc.vector.tensor_tensor(w4v[:, :, 1, 0], ly[:], wx0[:], AOT.mult)
                nc.vector.tensor_tensor(w4v[:, :, 2, 0], wy0[:], lx[:], AOT.mult)
                nc.vector.tensor_tensor(w4v[:, :, 3, 0], ly[:], lx[:], AOT.mult)
                qy = spool.tile([128, NB], dt.float32, tag="qy")
                nc.vector.tensor_scalar(qy[:], fy[:], 0.0, QCLAMP, AOT.max, AOT.min)
                qx = spool.tile([128, NB], dt.float32, tag="qx")
                nc.vector.tensor_scalar(qx[:], fx[:], 0.0, QCLAMP, AOT.max, AOT.min)
                qf = spool.tile([128, NB], dt.float32, tag="qf")
                nc.vector.scalar_tensor_tensor(qf[:], qy[:], float(PW), qx[:],
                                               AOT.mult, AOT.add)
                # idx construction: T1 then 8x T2 into 16-row staging
                t1p = ps_m.tile([NB, 128], dt.float32, tag="ps_misc")
                nc.tensor.transpose(t1p[:], qf[:], id_s[:])
                t1s = spool.tile([NB, 128], dt.float32, tag="t1s")
                nc.vector.tensor_copy(t1s[:], t1p[:])
                stv = idx_all[0:16, :].rearrange("p (k b r) -> p k b r", k=K2, r=8)
                for r in range(8):
                    t2p = ps_m.tile([16, NB], dt.float32, tag="ps_misc")
                    nc.tensor.transpose(t2p[:], t1s[:, 16 * r: 16 * r + 16],
                                        id_s[0:NB, 0:NB])
                    nc.any.tensor_copy(stv[:, k, :, r], t2p[:])
                # replicate rows 0..15 -> groups 1..7 for this tap
                for g in range(1, 8):
                    nc.sync.dma_start(
                        idx_all[16 * g: 16 * g + 16,
                                (HW // 16) * k: (HW // 16) * (k + 1)],
                        idx_all[0:16, (HW // 16) * k: (HW // 16) * (k + 1)])

            if STOP == "idx":
                nc.sync.dma_start(
                    out_d[0:64, 0:K2 * NB * 4].bitcast(dt.bfloat16)[:, 0:K2 * NB * 4],
                    w4_all[0:64, :])
                return

            # ---- 5. main loop: gather -> weight -> transpose -> conv ----
            DO_W = STOP not in ("g1",)
            DO_T = STOP in ("gt", "loop", "full")
            DO_MM = STOP in ("loop", "full")
            NCH_RUN = 1 if STOP in ("g1", "gw", "gt") else NCH
            x2win = bass.AP(x2pad.tensor, 0, [[128, NQ - 1], [1, 256]])
            conv_s = ppool.tile([C, HW], dt.float32)
            sums = ppool.tile([C, 8], dt.float32)
            sqs = ppool.tile([C, 8], dt.float32)
            gq = 0
            for ch in range(NCH_RUN):
                po = ps_o.tile([C, NPIX], dt.float32, tag="ps_out")
                for pr in range(NPAIR):
                    s2 = wpool.tile([128, CHUNK * 128], dt.bfloat16, tag="s2")
                    s2v = s2[:].rearrange("p (b h c) -> p b h c", h=2, c=C)
                    for h in range(2):
                        k = 2 * pr + h
                        if k == K2:
                            # dummy tap: stale bf16 data x zero lhsT rows
                            continue
                        g_t = gpool.tile([128, CHUNK * 256], dt.bfloat16, tag="g")
                        gview = g_t[:].rearrange("p (b e) -> p b e", e=256)
                        for g3 in range(3):
                            cbase = (HW // 16) * k + 96 * ch + 32 * g3
                            nc.gpsimd.dma_gather(
                                out_ap=gview[:, 4 * g3: 4 * g3 + 4, :],
                                in_ap=x2win,
                                idxs_ap=idx_all[:, cbase: cbase + 32],
                                num_idxs=512,
                                num_idxs_reg=512,
                                elem_size=256,
                                elem_step=128,
                                queue_num=gq % 4,
                            )
                            gq += 1
                        if not DO_W:
                            nc.sync.dma_start(
                                out_d[0:64, 0:CHUNK * 128].bitcast(dt.bfloat16)[:, 0:CHUNK * 256],
                                g_t[0:64, :])
                            return
                        gv = g_t[:].rearrange("p (b j c) -> p b j c", j=4, c=C)
                        sv = s2v[:, :, h, :]
                        # expand per-pixel weights over channels on the ACT
                        # engine so every DVE op below is bf16 step-1 packed
                        wexp = wpool.tile([128, CHUNK * 256], dt.bfloat16,
                                          tag="wexp")
                        wev = wexp[:].rearrange("p (b j c) -> p b j c", j=4, c=C)
                        wj = w4v_all[:, k, CHUNK * ch: CHUNK * ch + CHUNK, :, :]
                        e1, e2 = bass.broadcast_tensor_aps(wev, wj)
                        nc.scalar.copy(e1, e2)
                        wg = wpool.tile([128, CHUNK * 256], dt.bfloat16, tag="wg")
                        nc.vector.tensor_tensor(wg[:], g_t[:], wexp[:], AOT.mult)
                        wgv = wg[:].rearrange("p (b j c) -> p b j c", j=4, c=C)
                        tmp = wpool.tile([128, CHUNK * C], dt.bfloat16, tag="tmp")
                        tv = tmp[:].rearrange("p (b c) -> p b c", c=C)
                        nc.vector.tensor_tensor(tv, wgv[:, :, 0, :],
                                                wgv[:, :, 1, :], AOT.add)
                        tmp2 = wpool.tile([128, CHUNK * C], dt.bfloat16, tag="tmp2")
                        tv2 = tmp2[:].rearrange("p (b c) -> p b c", c=C)
                        nc.vector.tensor_tensor(tv2, wgv[:, :, 2, :],
                                                wgv[:, :, 3, :], AOT.add)
                        nc.vector.tensor_tensor(sv, tv, tv2, AOT.add)
                    if not DO_T:
                        if pr == NPAIR - 1:
                            nc.sync.dma_start(
                                out_d[0:64, 0:CHUNK * 64].bitcast(dt.bfloat16)[:, 0:CHUNK * 128],
                                s2[0:64, :])
                            return
                        continue
                    # transposes -> channel-major sampled (3 blocks per group);
                    # PSUM->SBUF copies alternate ACT/DVE to balance engines
                    samp = wpool.tile([128, CHUNK * 128], dt.bfloat16, tag="samp")
                    for grp in range(CHUNK // 3):
                        pt = ps_t.tile([128, 384], dt.bfloat16, tag="ps_tr")
                        for bb in range(3):
                            col = 384 * grp + 128 * bb
                            nc.tensor.transpose(pt[:, 128 * bb: 128 * bb + 128],
                                                s2[:, col: col + 128], idb_s[:])
                        dst = samp[:, 384 * grp: 384 * grp + 384]
                        if grp % 2 == 0:
                            nc.scalar.copy(dst, pt[:])
                        else:
                            nc.vector.tensor_copy(dst, pt[:])
                    if not DO_MM:
                        if pr == NPAIR - 1:
                            nc.sync.dma_start(
                                out_d[0:64, 0:CHUNK * 64].bitcast(dt.bfloat16)[:, 0:CHUNK * 128],
                                samp[0:64, :])
                            return
                        continue
                    # one 128-deep matmul per tap pair, natural pixel order
                    st, sp = (pr == 0), (pr == NPAIR - 1)
                    lh = wdefT_s[:, C * pr: C * pr + C]
                    for j3 in range(3):
                        nc.tensor.matmul(po[:, 512 * j3: 512 * j3 + 512], lh,
                                         samp[:, 512 * j3: 512 * j3 + 512],
                                         start=st, stop=sp)
                # copy conv chunk to SBUF + per-chunk sum / sumsq
                cview = conv_s[:, NPIX * ch: NPIX * ch + NPIX]
                nc.scalar.activation(cview, po[:], AFT.Copy,
                                     accum_out=sums[:, ch: ch + 1])
                scr = wpool.tile([C, NPIX], dt.float32, tag="scr", bufs=1)
                nc.scalar.activation(scr[:], cview, AFT.Square,
                                     accum_out=sqs[:, ch: ch + 1])

            if STOP == "loop":
                nc.sync.dma_start(out_d[:], conv_s[:])
                return

            # ---- 6. BN stats allreduce + normalize + relu + output ----
            st2 = ppool.tile([C, 2], dt.float32)
            nc.vector.tensor_reduce(st2[:, 0:1], sums[:, 0:NCH],
                                    mybir.AxisListType.X, AOT.add)
            nc.vector.tensor_reduce(st2[:, 1:2], sqs[:, 0:NCH],
                                    mybir.AxisListType.X, AOT.add)
            bi = dpool.tile([C, 2], dt.float32)
            bo = dpool.tile([C, 2], dt.float32)
            nc.gpsimd.dma_start(bi[:], st2[:])
            if use_collective:
                nc.gpsimd.collective_compute(
                    "AllReduce", AOT.add,
                    replica_groups=[list(range(ncores))],
                    ins=[bi.opt()], outs=[bo.opt()])
            else:
                nc.gpsimd.dma_start(bo[:], bi[:])
            ast = ppool.tile([C, 2], dt.float32)
            nc.gpsimd.dma_start(ast[:], bo[:])

            inv_n = 1.0 / float(ncores * HW)
            mean = ppool.tile([C, 1], dt.float32)
            nc.vector.tensor_scalar(mean[:], ast[:, 0:1], inv_n, None, AOT.mult)
            msq = ppool.tile([C, 1], dt.float32)
            nc.vector.tensor_scalar(msq[:], ast[:, 1:2], inv_n, None, AOT.mult)
            m2 = ppool.tile([C, 1], dt.float32)
            nc.vector.tensor_tensor(m2[:], mean[:], mean[:], AOT.mult)
            var = ppool.tile([C, 1], dt.float32)
            nc.vector.tensor_tensor(var[:], msq[:], m2[:], AOT.subtract)
            vare = ppool.tile([C, 1], dt.float32)
            nc.vector.tensor_scalar(vare[:], var[:], EPS, None, AOT.add)
            sd = ppool.tile([C, 1], dt.float32)
            nc.scalar.activation(sd[:], vare[:], AFT.Sqrt)
            inv = ppool.tile([C, 1], dt.float32)
            nc.vector.reciprocal(inv[:], sd[:])
            scl = ppool.tile([C, 1], dt.float32)
            nc.vector.tensor_tensor(scl[:], bnc_s[:, 0:1], inv[:], AOT.mult)
            mt = ppool.tile([C, 1], dt.float32)
            nc.vector.tensor_tensor(mt[:], mean[:], scl[:], AOT.mult)
            bia = ppool.tile([C, 1], dt.float32)
            nc.vector.tensor_tensor(bia[:], bnc_s[:, 1:2], mt[:], AOT.subtract)

            for ch in range(NCH):
                on = wpool.tile([C, NPIX], dt.float32, tag="on")
                nc.scalar.activation(on[:], conv_s[:, NPIX * ch: NPIX * ch + NPIX],
                                     AFT.Relu, bias=bia[:], scale=scl[:])
                nc.sync.dma_start(out_d[:, NPIX * ch: NPIX * ch + NPIX], on[:])


def _prep_core(xb, w_off, b_off, w_def, gamma, beta):
    """Host-side input prep for one batch item. xb: [64, 96, 96] f32."""
    ins = {}
    # xpad: zero-pad for the 3x3 offset conv; 99 rows so the ky-shifted
    # bottom half's 6-row DMA stays in range for the last chunk
    xp = np.zeros((C, 99, 98), np.float32)
    xp[:, 1:97, 1:97] = xb
    ins["xpad"] = xp.reshape(C, 99 * 98)
    # x2pad: padded HWC grid (bf16), rows [xz[r,s,:], xz[r+1,s,:]]
    xz = np.zeros((PR + 1, PW, C), np.float32)
    xz[PADM:PADM + H, PADM:PADM + W] = xb.transpose(1, 2, 0)
    xzf = xz.reshape((PR + 1) * PW, C)
    x2 = np.concatenate([xzf[0:NQ], xzf[PW:NQ + PW]], axis=1)
    ins["x2pad"] = x2.astype(ml_dtypes.bfloat16)
    # offset conv weights: cols kx 0..2 = vertical pair (ky0+ky1), 3+kx = ky2
    wofft = np.zeros((128, 6 * 18), np.float32)
    for kx in range(K):
        wofft[0:C, 18 * kx:18 * kx + 18] = w_off[:, :, 0, kx].T
        wofft[C:128, 18 * kx:18 * kx + 18] = w_off[:, :, 1, kx].T
        wofft[0:C, 18 * (3 + kx):18 * (3 + kx) + 18] = w_off[:, :, 2, kx].T
    ins["woffT"] = wofft
    # deformable conv weights: tap pairs stacked over 128 contraction rows
    wdeft = np.zeros((128, NPAIR * C), np.float32)
    for pr in range(NPAIR):
        for h in range(2):
            k = 2 * pr + h
            if k < K2:
                wdeft[C * h:C * h + C, C * pr:C * pr + C] = \
                    w_def[:, :, k // K, k % K].T
    ins["wdefT"] = wdeft.astype(ml_dtypes.bfloat16)
    # base grids (pixel-major [128, 72] per tap), fold b_off and pad margin
    pixi = np.arange(HW, dtype=np.int64)
    ygrid = (pixi // W).astype(np.float32)
    xgrid = (pixi % W).astype(np.float32)
    ypm = ygrid.reshape(NB, 128).T    # [p, b] pixel-major
    xpm = xgrid.reshape(NB, 128).T
    bpy = np.zeros((128, K2 * NB), np.float32)
    bpx = np.zeros((128, K2 * NB), np.float32)
    for k in range(K2):
        ky, kx = k // K - 1, k % K - 1
        bpy[:, NB * k:NB * k + NB] = ypm + (ky + PADM + b_off[2 * k])
        bpx[:, NB * k:NB * k + NB] = xpm + (kx + PADM + b_off[2 * k + 1])
    ins["bpy"] = bpy
    ins["bpx"] = bpx
    ins["ident"] = np.eye(128, dtype=np.float32)
    ins["identb"] = np.eye(128, dtype=np.float32).astype(ml_dtypes.bfloat16)
    ins["bnc"] = np.stack([gamma, beta], axis=1).astype(np.float32)
    return ins


def _get_nc():
    if "nc" not in _CACHE:
        nc = bacc.Bacc("TRN2", target_bir_lowering=False, debug=False,
                       num_devices=NCORES, num_swdge_queues=4)
        _build(nc)
        nc.compile()
        _CACHE["nc"] = nc
    return _CACHE["nc"]


def kernel(x, w_off, b_off, w_def, b_def, gamma, beta, trace=False):
    x = np.asarray(x, np.float32)
    w_off = np.asarray(w_off, np.float32)
    b_off = np.asarray(b_off, np.float32)
    w_def = np.asarray(w_def, np.float32)
    gamma = np.asarray(gamma, np.float32)
    beta = np.asarray(beta, np.float32)
    # b_def cancels exactly in training-mode BN; accepted but unused.
    nc = _get_nc()
    in_maps = [_prep_core(x[b], w_off, b_off, w_def, gamma, beta)
               for b in range(B)]
    res = bass_utils.run_bass_kernel_spmd(
        nc, in_maps, core_ids=list(range(NCORES)), trace=trace)
    out = np.stack([res.results[b]["out"].reshape(C, H, W) for b in range(B)])
    if trace:
        kernel.last_exec_time_ns = res.exec_time_ns
        kernel.last_results = res
    return out
